# revision 9
# baseline (speedup 1.0000x reference)
"""SRU (Simple Recurrent Unit) 2-layer network + FC head on 8 Trainium2 cores.

Model (per reference):
  U = x @ W          -> xt, fi, ri gates            [L, B, 3H]
  f = sigmoid(fi+bf); r = sigmoid(ri+br)
  c_t = f_t * c_{t-1} + (1-f_t) * xt_t              (scan over L)
  h = r * c + (1-r) * x                             (highway)
  out = h2[last] @ Wfc + bfc

Sharding: data-parallel over batch (B=32 -> 4 per core), weights replicated.

Per-core kernel layout strategy: compute U^T per batch in [3H, L] layout
(W chunk stationary, x^T moving) so time lies along the SBUF free dim --
the hardware tensor_tensor_scan instruction then runs the recurrence
(state = f*state + g) 128 lanes at a time.  The scan/highway output [H, L]
is directly the moving operand of the next layer's GEMM, so only the input
x needs an on-chip transpose (PE transpose via identity).  Layer 2 only
needs its r gate at the last timestep (only h2[-1] feeds the FC head), so
its ri GEMM is a single-column matvec and the full highway is skipped.
"""

import os
import sys
import tempfile
from contextlib import ExitStack
from dataclasses import dataclass

import numpy as np

try:
    import concourse.bass as bass
except ImportError:
    sys.path.insert(0, "/opt/trn_rl_repo")
    import concourse.bass as bass

import concourse.mybir as mybir
import concourse.tile as tile
from concourse import bacc
from concourse.bass import ds, ts
from concourse.bass_utils import run_bass_kernel_spmd
from concourse.masks import make_identity

P = 128
F32 = mybir.dt.float32
BF16 = mybir.dt.bfloat16
AF = mybir.ActivationFunctionType
OP = mybir.AluOpType


@dataclass(frozen=True)
class Cfg:
    L: int = 1024          # timesteps
    BL: int = 4            # batch per core
    D: int = 768           # input dim (== H)
    H: int = 768           # hidden dim
    C: int = 1000          # classes
    n_cores: int = 8
    dma_transpose: bool = True   # x^T via DMA xbar (vs PE transpose via identity)

    @property
    def KC(self):          # contraction chunks of 128
        return self.D // P

    @property
    def HC(self):          # hidden chunks of 128
        return self.H // P

    @property
    def LB(self):          # timestep blocks of 128
        return self.L // P

    @property
    def HALF(self):        # PSUM-bank-sized slice of L
        return min(self.L, 512)

    @property
    def NHALF(self):
        return self.L // self.HALF


def _build_body(ctx, tc, cfg, aps):
    nc = tc.nc
    L, BL, D, H, C = cfg.L, cfg.BL, cfg.D, cfg.H, cfg.C
    KC, HC, LB, HALF, NHALF = cfg.KC, cfg.HC, cfg.LB, cfg.HALF, cfg.NHALF
    x, w1, bf1, br1, w2, bf2, br2, wfc, bfc, out = aps

    singles = ctx.enter_context(tc.tile_pool(name="singles", bufs=1))
    wstage = ctx.enter_context(tc.tile_pool(name="wstage", bufs=4))
    xstage = ctx.enter_context(tc.tile_pool(name="xstage", bufs=3))
    xbfp = ctx.enter_context(tc.tile_pool(name="xbf", bufs=3))
    xTpool = ctx.enter_context(tc.tile_pool(name="xT", bufs=2 * KC))
    h1pool = ctx.enter_context(tc.tile_pool(name="h1", bufs=2 * KC))
    gates = ctx.enter_context(tc.tile_pool(name="gates", bufs=2))
    csp = ctx.enter_context(tc.tile_pool(name="cs", bufs=2))
    smalls = ctx.enter_context(tc.tile_pool(name="smalls", bufs=4))
    ps_g = ctx.enter_context(tc.tile_pool(name="ps_g", bufs=2, space="PSUM"))
    if not cfg.dma_transpose:
        ps_tp = ctx.enter_context(tc.tile_pool(name="ps_tp", bufs=1, space="PSUM"))
    ps_misc = ctx.enter_context(tc.tile_pool(name="ps_misc", bufs=1, space="PSUM"))

    if not cfg.dma_transpose:
        ident = singles.tile([P, P], F32, tag="ident", name="ident")
        make_identity(nc, ident)

    # ---- weights: DRAM f32 -> SBUF bf16 ----
    # cast engine split: W1 on ACT (needed first, ACT idle at start), W2 on
    # DVE, Wfc on POOL -- keeps PE's first matmuls unblocked by the casts.
    def load_weight_bf16(wap, ncols, nm, cast):
        tiles = []
        for kc in range(KC):
            wt = singles.tile([P, ncols], BF16, tag=f"{nm}_{kc}", name=f"{nm}_{kc}")
            half = ncols // 2
            for ch in range(2):
                csl = ds(ch * half, half)
                st = wstage.tile([P, half], F32, tag="wstage", name="wstage")
                nc.sync.dma_start(out=st, in_=wap[ts(kc, P), csl])
                cast(wt[:, csl], st)
            tiles.append(wt)
        return tiles

    w1bf = load_weight_bf16(w1, 3 * H, "w1", nc.scalar.copy)
    w2bf = load_weight_bf16(w2, 3 * H, "w2", nc.vector.tensor_copy)
    wfcbf = load_weight_bf16(wfc, C, "wfc", nc.gpsimd.tensor_copy)

    # ---- biases as [P, HC] column tiles (+ negated for 1-sigmoid trick) ----
    def load_bias(vec, nm):
        t = singles.tile([P, HC], F32, tag=f"b_{nm}", name=f"b_{nm}")
        nt = singles.tile([P, HC], F32, tag=f"nb_{nm}", name=f"nb_{nm}")
        nc.sync.dma_start(out=t, in_=vec.rearrange("(c p) -> p c", p=P))
        nc.scalar.mul(nt, t, -1.0)
        return t, nt

    bf1_sb, nbf1_sb = load_bias(bf1, "f1")
    br1_sb, nbr1_sb = load_bias(br1, "r1")
    bf2_sb, nbf2_sb = load_bias(bf2, "f2")
    br2_sb, nbr2_sb = load_bias(br2, "r2")

    bfc_f32 = singles.tile([1, C], F32, tag="bfc32", name="bfc32")
    nc.sync.dma_start(out=bfc_f32, in_=bfc[None, :])
    bfc_bf = singles.tile([1, C], BF16, tag="bfc16", name="bfc16")
    nc.gpsimd.tensor_copy(bfc_bf, bfc_f32)
    ones_bf = singles.tile([1, BL], BF16, tag="ones", name="ones")
    nc.vector.memset(ones_bf, 1.0)

    last_bf = [singles.tile([P, BL], BF16, tag=f"last_{hc}", name=f"last_{hc}") for hc in range(HC)]

    # ---- x[b] -> x^T in bf16 ----
    def transpose_batch(b):
        if cfg.dma_transpose:
            # load f32 -> ACT cast to bf16 -> DMA xbar transpose [128,128] blocks
            xT = [xTpool.tile([P, L], BF16, tag="xT", name="xT")
                  for _ in range(KC)]
            for lb in range(LB):
                t = xstage.tile([P, D], F32, tag="xs", name="xs")
                nc.sync.dma_start(out=t, in_=x[b, ts(lb, P), :])
                xb = xbfp.tile([P, D], BF16, tag="xbf", name="xbf")
                nc.scalar.copy(xb, t)
                for dc in range(KC):
                    nc.sync.dma_start_transpose(
                        out=xT[dc][:, ts(lb, P)], in_=xb[:, ts(dc, P)])
            return xT
        xs = []
        for lb in range(LB):
            t = xstage.tile([P, D], F32, tag="xs", name="xs", bufs=LB)
            nc.sync.dma_start(out=t, in_=x[b, ts(lb, P), :])
            xs.append(t)
        xT = []
        for dc in range(KC):
            xt_t = xTpool.tile([P, L], BF16, tag="xT", name="xT")
            for half in range(NHALF):
                pt = ps_tp.tile([P, HALF], F32, tag="tp", name="tp")
                for j in range(HALF // P):
                    lb = half * (HALF // P) + j
                    nc.tensor.transpose(pt[:, ts(j, P)], xs[lb][:, ts(dc, P)], ident)
                nc.scalar.copy(xt_t[:, ds(half * HALF, HALF)], pt)
            xT.append(xt_t)
        return xT

    def gate_matmuls(ps, wbf, gate, hc, rhs_tiles, lsl):
        for kc in range(KC):
            nc.tensor.matmul(
                ps,
                wbf[kc][:, ds(gate * H + hc * P, P)],
                rhs_tiles[kc][:, lsl],
                start=kc == 0,
                stop=kc == KC - 1,
            )

    # ---- SRU layer 1: full highway, h1 in [H, L] bf16 ----
    def layer1(b, xT):
        h1 = []
        for hc in range(HC):
            f_t = gates.tile([P, L], F32, tag="f", name="f")
            omf_t = gates.tile([P, L], F32, tag="omf", name="omf")
            r_t = gates.tile([P, L], F32, tag="r", name="r")
            omr_t = gates.tile([P, L], F32, tag="omr", name="omr")
            cs_t = csp.tile([P, L], F32, tag="cs", name="cs")
            for half in range(NHALF):
                lsl = ds(half * HALF, HALF)
                ps_xt = ps_g.tile([P, HALF], F32, tag="ps_xt", name="ps_xt",
                                  bufs=3 if cfg.dma_transpose else 2)
                ps_fi = ps_g.tile([P, HALF], F32, tag="ps_fi", name="ps_fi")
                ps_ri = ps_g.tile([P, HALF], F32, tag="ps_ri", name="ps_ri")
                gate_matmuls(ps_xt, w1bf, 0, hc, xT, lsl)
                gate_matmuls(ps_fi, w1bf, 1, hc, xT, lsl)
                gate_matmuls(ps_ri, w1bf, 2, hc, xT, lsl)
                hsl = ds(hc, 1)
                nc.scalar.activation(f_t[:, lsl], ps_fi, AF.Sigmoid,
                                     bias=bf1_sb[:, hsl], scale=1.0)
                nc.scalar.activation(omf_t[:, lsl], ps_fi, AF.Sigmoid,
                                     bias=nbf1_sb[:, hsl], scale=-1.0)
                nc.scalar.activation(r_t[:, lsl], ps_ri, AF.Sigmoid,
                                     bias=br1_sb[:, hsl], scale=1.0)
                nc.scalar.activation(omr_t[:, lsl], ps_ri, AF.Sigmoid,
                                     bias=nbr1_sb[:, hsl], scale=-1.0)
                # g = (1-f) * xt, in place over omf
                nc.vector.tensor_mul(omf_t[:, lsl], omf_t[:, lsl], ps_xt)
            # recurrence: c = f*c + g  (chained scan over L halves)
            nc.vector.tensor_tensor_scan(
                cs_t[:, 0:HALF], f_t[:, 0:HALF], omf_t[:, 0:HALF],
                0.0, OP.mult, OP.add)
            for half in range(1, NHALF):
                lsl = ds(half * HALF, HALF)
                nc.vector.tensor_tensor_scan(
                    cs_t[:, lsl], f_t[:, lsl], omf_t[:, lsl],
                    cs_t[:, half * HALF - 1:half * HALF], OP.mult, OP.add)
            # highway: h1 = r*cs + (1-r)*x
            nc.vector.tensor_mul(r_t, r_t, cs_t)
            nc.gpsimd.tensor_mul(omr_t, omr_t, xT[hc])
            h1_t = h1pool.tile([P, L], BF16, tag="h1", name="h1")
            nc.gpsimd.tensor_add(h1_t, r_t, omr_t)
            h1.append(h1_t)
        return h1

    # ---- SRU layer 2: highway only at last timestep -> last_bf columns ----
    def layer2(b, h1):
        for hc in range(HC):
            f_t = gates.tile([P, L], F32, tag="f", name="f")
            omf_t = gates.tile([P, L], F32, tag="omf", name="omf")
            cs_t = csp.tile([P, L], F32, tag="cs", name="cs")
            for half in range(NHALF):
                lsl = ds(half * HALF, HALF)
                ps_xt = ps_g.tile([P, HALF], F32, tag="ps_xt", name="ps_xt",
                                  bufs=3 if cfg.dma_transpose else 2)
                ps_fi = ps_g.tile([P, HALF], F32, tag="ps_fi", name="ps_fi")
                gate_matmuls(ps_xt, w2bf, 0, hc, h1, lsl)
                gate_matmuls(ps_fi, w2bf, 1, hc, h1, lsl)
                hsl = ds(hc, 1)
                nc.scalar.activation(f_t[:, lsl], ps_fi, AF.Sigmoid,
                                     bias=bf2_sb[:, hsl], scale=1.0)
                nc.scalar.activation(omf_t[:, lsl], ps_fi, AF.Sigmoid,
                                     bias=nbf2_sb[:, hsl], scale=-1.0)
                nc.vector.tensor_mul(omf_t[:, lsl], omf_t[:, lsl], ps_xt)
            nc.vector.tensor_tensor_scan(
                cs_t[:, 0:HALF], f_t[:, 0:HALF], omf_t[:, 0:HALF],
                0.0, OP.mult, OP.add)
            for half in range(1, NHALF):
                lsl = ds(half * HALF, HALF)
                nc.vector.tensor_tensor_scan(
                    cs_t[:, lsl], f_t[:, lsl], omf_t[:, lsl],
                    cs_t[:, half * HALF - 1:half * HALF], OP.mult, OP.add)
            # r2 only needed at t = L-1
            ps_ri2 = ps_misc.tile([P, 1], F32, tag="misc", name="misc")
            for kc in range(KC):
                nc.tensor.matmul(
                    ps_ri2, w2bf[kc][:, ds(2 * H + hc * P, P)],
                    h1[kc][:, L - 1:L], start=kc == 0, stop=kc == KC - 1)
            r2 = smalls.tile([P, 1], F32, tag="r2", name="r2")
            omr2 = smalls.tile([P, 1], F32, tag="omr2", name="omr2")
            hsl = ds(hc, 1)
            nc.scalar.activation(r2, ps_ri2, AF.Sigmoid,
                                 bias=br2_sb[:, hsl], scale=1.0)
            nc.scalar.activation(omr2, ps_ri2, AF.Sigmoid,
                                 bias=nbr2_sb[:, hsl], scale=-1.0)
            nc.vector.tensor_mul(r2, r2, cs_t[:, L - 1:L])
            nc.vector.tensor_mul(omr2, omr2, h1[hc][:, L - 1:L])
            nc.vector.tensor_add(last_bf[hc][:, ds(b, 1)], r2, omr2)

    for b in range(BL):
        xT = transpose_batch(b)
        h1 = layer1(b, xT)
        layer2(b, h1)

    # ---- FC head: out[b, c] = sum_h last[h, b] * Wfc[h, c] + bfc[c] ----
    out_sb = singles.tile([BL, C], F32, tag="out_sb", name="out_sb")
    chalf = C // 2
    for nh in range(2):
        csl = ds(nh * chalf, chalf)
        ps_fc = ps_misc.tile([BL, chalf], F32, tag="misc", name="misc")
        for kc in range(KC):
            nc.tensor.matmul(ps_fc, last_bf[kc], wfcbf[kc][:, csl],
                             start=kc == 0, stop=False)
        nc.tensor.matmul(ps_fc, ones_bf, bfc_bf[:, csl], start=False, stop=True)
        nc.scalar.copy(out_sb[:, csl], ps_fc)
    nc.sync.dma_start(out=out, in_=out_sb)


def build_program(cfg: Cfg = Cfg()):
    nc = bacc.Bacc("TRN2", target_bir_lowering=False, num_devices=cfg.n_cores)
    x = nc.dram_tensor("x", [cfg.BL, cfg.L, cfg.D], F32, kind="ExternalInput").ap()
    w1 = nc.dram_tensor("w1", [cfg.D, 3 * cfg.H], F32, kind="ExternalInput").ap()
    bf1 = nc.dram_tensor("bf1", [cfg.H], F32, kind="ExternalInput").ap()
    br1 = nc.dram_tensor("br1", [cfg.H], F32, kind="ExternalInput").ap()
    w2 = nc.dram_tensor("w2", [cfg.H, 3 * cfg.H], F32, kind="ExternalInput").ap()
    bf2 = nc.dram_tensor("bf2", [cfg.H], F32, kind="ExternalInput").ap()
    br2 = nc.dram_tensor("br2", [cfg.H], F32, kind="ExternalInput").ap()
    wfc = nc.dram_tensor("wfc", [cfg.H, cfg.C], F32, kind="ExternalInput").ap()
    bfc = nc.dram_tensor("bfc", [cfg.C], F32, kind="ExternalInput").ap()
    out = nc.dram_tensor("out", [cfg.BL, cfg.C], F32, kind="ExternalOutput").ap()
    aps = (x, w1, bf1, br1, w2, bf2, br2, wfc, bfc, out)
    with tile.TileContext(nc) as tc:
        with ExitStack() as ctx:
            _build_body(ctx, tc, cfg, aps)
    nc.compile()
    return nc


def make_in_maps(inputs, cfg: Cfg = Cfg()):
    """Shard the full inputs: batch-slice x per core, replicate weights."""
    x = np.ascontiguousarray(inputs["x"])  # [L, B, D]
    shared = {
        k: np.ascontiguousarray(np.asarray(inputs[k], dtype=np.float32))
        for k in ("W1", "bf1", "br1", "W2", "bf2", "br2", "Wfc", "bfc")
    }
    rename = {"W1": "w1", "W2": "w2", "Wfc": "wfc"}
    shared = {rename.get(k, k): v for k, v in shared.items()}
    in_maps = []
    for core in range(cfg.n_cores):
        bsl = slice(core * cfg.BL, (core + 1) * cfg.BL)
        # [L, BL, D] -> [BL, L, D] contiguous, best DMA layout
        x_loc = np.ascontiguousarray(
            x[:, bsl, :].transpose(1, 0, 2).astype(np.float32))
        in_maps.append({"x": x_loc, **shared})
    return in_maps


_CACHE = {}


def kernel(**inputs) -> np.ndarray:
    cfg = Cfg()
    if "nc" not in _CACHE:
        _CACHE["nc"] = build_program(cfg)
    nc = _CACHE["nc"]
    in_maps = make_in_maps(inputs, cfg)
    res = run_bass_kernel_spmd(nc, in_maps, core_ids=list(range(cfg.n_cores)))
    outs = [res.results[c]["out"] for c in range(cfg.n_cores)]
    return np.concatenate(outs, axis=0).astype(np.float32)


if __name__ == "__main__":
    rng = np.random.default_rng(0)
    cfg = Cfg()
    fake = {
        "x": rng.standard_normal((cfg.L, 8 * cfg.BL, cfg.D), dtype=np.float32),
        "W1": rng.standard_normal((cfg.D, 3 * cfg.H), dtype=np.float32) * 0.02,
        "bf1": np.zeros(cfg.H, np.float32),
        "br1": np.zeros(cfg.H, np.float32),
        "W2": rng.standard_normal((cfg.H, 3 * cfg.H), dtype=np.float32) * 0.02,
        "bf2": np.zeros(cfg.H, np.float32),
        "br2": np.zeros(cfg.H, np.float32),
        "Wfc": rng.standard_normal((cfg.H, cfg.C), dtype=np.float32) * 0.02,
        "bfc": np.zeros(cfg.C, np.float32),
    }
    out = kernel(**fake)
    print("kernel output", out.shape, out.dtype)


# revision 10
# speedup vs baseline: 1.3934x; 1.3934x over previous
"""SRU (Simple Recurrent Unit) 2-layer network + FC head on 8 Trainium2 cores.

Model (per reference):
  U = x @ W          -> xt, fi, ri gates            [L, B, 3H]
  f = sigmoid(fi+bf); r = sigmoid(ri+br)
  c_t = f_t * c_{t-1} + (1-f_t) * xt_t              (scan over L)
  h = r * c + (1-r) * x                             (highway)
  out = h2[last] @ Wfc + bfc

Sharding: data-parallel over batch (B=32 -> 4 per core), weights replicated.

Per-core kernel layout strategy: compute U^T per batch in [3H, L] layout
(W chunk stationary, x^T moving) so time lies along the SBUF free dim --
the hardware tensor_tensor_scan instruction then runs the recurrence
(state = f*state + g) 128 lanes at a time.  The scan/highway output [H, L]
is directly the moving operand of the next layer's GEMM, so only the input
x needs an on-chip transpose (PE transpose via identity).  Layer 2 only
needs its r gate at the last timestep (only h2[-1] feeds the FC head), so
its ri GEMM is a single-column matvec and the full highway is skipped.
"""

import os
import sys
import tempfile
from contextlib import ExitStack
from dataclasses import dataclass

import numpy as np

try:
    import concourse.bass as bass
except ImportError:
    sys.path.insert(0, "/opt/trn_rl_repo")
    import concourse.bass as bass

import concourse.mybir as mybir
import concourse.tile as tile
from concourse import bacc
from concourse.bass import ds, ts
from concourse.bass_utils import run_bass_kernel_spmd
from concourse.masks import make_identity

P = 128
F32 = mybir.dt.float32
BF16 = mybir.dt.bfloat16
AF = mybir.ActivationFunctionType
OP = mybir.AluOpType


@dataclass(frozen=True)
class Cfg:
    L: int = 1024          # timesteps
    BL: int = 4            # batch per core
    D: int = 768           # input dim (== H)
    H: int = 768           # hidden dim
    C: int = 1000          # classes
    n_cores: int = 8
    dma_transpose: bool = False  # x^T via DMA xbar (vs PE transpose via identity)
                                 # measured: DMA xbar path serializes the sync
                                 # engine (~1.2us/block) -> 700us, keep PE path

    @property
    def KC(self):          # contraction chunks of 128
        return self.D // P

    @property
    def HC(self):          # hidden chunks of 128
        return self.H // P

    @property
    def LB(self):          # timestep blocks of 128
        return self.L // P

    @property
    def HALF(self):        # PSUM-bank-sized slice of L
        return min(self.L, 512)

    @property
    def NHALF(self):
        return self.L // self.HALF


def _build_body(ctx, tc, cfg, aps):
    nc = tc.nc
    L, BL, D, H, C = cfg.L, cfg.BL, cfg.D, cfg.H, cfg.C
    KC, HC, LB, HALF, NHALF = cfg.KC, cfg.HC, cfg.LB, cfg.HALF, cfg.NHALF
    x, w1, bf1, br1, w2, bf2, br2, wfc, bfc, out = aps

    singles = ctx.enter_context(tc.tile_pool(name="singles", bufs=1))
    wstage = ctx.enter_context(tc.tile_pool(name="wstage", bufs=4))
    xstage = ctx.enter_context(tc.tile_pool(name="xstage", bufs=3))
    xbfp = ctx.enter_context(tc.tile_pool(name="xbf", bufs=3))
    xTpool = ctx.enter_context(tc.tile_pool(name="xT", bufs=2 * KC))
    h1pool = ctx.enter_context(tc.tile_pool(name="h1", bufs=2 * KC))
    gates = ctx.enter_context(tc.tile_pool(name="gates", bufs=2))
    csp = ctx.enter_context(tc.tile_pool(name="cs", bufs=2))
    smalls = ctx.enter_context(tc.tile_pool(name="smalls", bufs=4))
    ps_g = ctx.enter_context(tc.tile_pool(name="ps_g", bufs=2, space="PSUM"))
    if not cfg.dma_transpose:
        ps_tp = ctx.enter_context(tc.tile_pool(name="ps_tp", bufs=1, space="PSUM"))
    ps_misc = ctx.enter_context(tc.tile_pool(name="ps_misc", bufs=1, space="PSUM"))

    if not cfg.dma_transpose:
        ident = singles.tile([P, P], F32, tag="ident", name="ident")
        make_identity(nc, ident)

    # ---- weights: DRAM f32 -> SBUF bf16 ----
    # cast engine split: W1 on ACT (needed first, ACT idle at start), W2 on
    # DVE, Wfc on POOL -- keeps PE's first matmuls unblocked by the casts.
    def load_weight_bf16(wap, ncols, nm, cast):
        tiles = []
        for kc in range(KC):
            wt = singles.tile([P, ncols], BF16, tag=f"{nm}_{kc}", name=f"{nm}_{kc}")
            half = ncols // 2
            for ch in range(2):
                csl = ds(ch * half, half)
                st = wstage.tile([P, half], F32, tag="wstage", name="wstage")
                nc.sync.dma_start(out=st, in_=wap[ts(kc, P), csl])
                cast(wt[:, csl], st)
            tiles.append(wt)
        return tiles

    w1bf = load_weight_bf16(w1, 3 * H, "w1", nc.scalar.copy)
    w2bf = load_weight_bf16(w2, 3 * H, "w2", nc.vector.tensor_copy)
    wfcbf = load_weight_bf16(wfc, C, "wfc", nc.gpsimd.tensor_copy)

    # ---- biases as [P, HC] column tiles (+ negated for 1-sigmoid trick) ----
    def load_bias(vec, nm):
        t = singles.tile([P, HC], F32, tag=f"b_{nm}", name=f"b_{nm}")
        nt = singles.tile([P, HC], F32, tag=f"nb_{nm}", name=f"nb_{nm}")
        nc.sync.dma_start(out=t, in_=vec.rearrange("(c p) -> p c", p=P))
        nc.scalar.mul(nt, t, -1.0)
        return t, nt

    bf1_sb, nbf1_sb = load_bias(bf1, "f1")
    br1_sb, nbr1_sb = load_bias(br1, "r1")
    bf2_sb, nbf2_sb = load_bias(bf2, "f2")
    br2_sb, nbr2_sb = load_bias(br2, "r2")

    bfc_f32 = singles.tile([1, C], F32, tag="bfc32", name="bfc32")
    nc.sync.dma_start(out=bfc_f32, in_=bfc[None, :])
    bfc_bf = singles.tile([1, C], BF16, tag="bfc16", name="bfc16")
    nc.gpsimd.tensor_copy(bfc_bf, bfc_f32)
    ones_bf = singles.tile([1, BL], BF16, tag="ones", name="ones")
    nc.vector.memset(ones_bf, 1.0)

    last_bf = [singles.tile([P, BL], BF16, tag=f"last_{hc}", name=f"last_{hc}") for hc in range(HC)]

    # ---- x[b] -> x^T in bf16 ----
    def transpose_batch(b):
        if cfg.dma_transpose:
            # load f32 -> ACT cast to bf16 -> DMA xbar transpose [128,128] blocks
            xT = [xTpool.tile([P, L], BF16, tag="xT", name="xT")
                  for _ in range(KC)]
            for lb in range(LB):
                t = xstage.tile([P, D], F32, tag="xs", name="xs")
                nc.sync.dma_start(out=t, in_=x[b, ts(lb, P), :])
                xb = xbfp.tile([P, D], BF16, tag="xbf", name="xbf")
                nc.scalar.copy(xb, t)
                for dc in range(KC):
                    nc.sync.dma_start_transpose(
                        out=xT[dc][:, ts(lb, P)], in_=xb[:, ts(dc, P)])
            return xT
        xs = []
        for lb in range(LB):
            t = xstage.tile([P, D], F32, tag="xs", name="xs", bufs=LB)
            nc.sync.dma_start(out=t, in_=x[b, ts(lb, P), :])
            xs.append(t)
        xT = []
        for dc in range(KC):
            xt_t = xTpool.tile([P, L], BF16, tag="xT", name="xT")
            for half in range(NHALF):
                pt = ps_tp.tile([P, HALF], F32, tag="tp", name="tp")
                for j in range(HALF // P):
                    lb = half * (HALF // P) + j
                    nc.tensor.transpose(pt[:, ts(j, P)], xs[lb][:, ts(dc, P)], ident)
                nc.scalar.copy(xt_t[:, ds(half * HALF, HALF)], pt)
            xT.append(xt_t)
        return xT

    def gate_matmuls(ps, wbf, gate, hc, rhs_tiles, lsl):
        for kc in range(KC):
            nc.tensor.matmul(
                ps,
                wbf[kc][:, ds(gate * H + hc * P, P)],
                rhs_tiles[kc][:, lsl],
                start=kc == 0,
                stop=kc == KC - 1,
            )

    # ---- SRU layer 1: full highway, h1 in [H, L] bf16 ----
    def layer1(b, xT):
        h1 = []
        for hc in range(HC):
            f_t = gates.tile([P, L], F32, tag="f", name="f")
            omf_t = gates.tile([P, L], F32, tag="omf", name="omf")
            r_t = gates.tile([P, L], F32, tag="r", name="r")
            omr_t = gates.tile([P, L], F32, tag="omr", name="omr")
            cs_t = csp.tile([P, L], F32, tag="cs", name="cs")
            for half in range(NHALF):
                lsl = ds(half * HALF, HALF)
                ps_xt = ps_g.tile([P, HALF], F32, tag="ps_xt", name="ps_xt",
                                  bufs=3 if cfg.dma_transpose else 2)
                ps_fi = ps_g.tile([P, HALF], F32, tag="ps_fi", name="ps_fi")
                ps_ri = ps_g.tile([P, HALF], F32, tag="ps_ri", name="ps_ri")
                gate_matmuls(ps_xt, w1bf, 0, hc, xT, lsl)
                gate_matmuls(ps_fi, w1bf, 1, hc, xT, lsl)
                gate_matmuls(ps_ri, w1bf, 2, hc, xT, lsl)
                hsl = ds(hc, 1)
                nc.scalar.activation(f_t[:, lsl], ps_fi, AF.Sigmoid,
                                     bias=bf1_sb[:, hsl], scale=1.0)
                nc.scalar.activation(omf_t[:, lsl], ps_fi, AF.Sigmoid,
                                     bias=nbf1_sb[:, hsl], scale=-1.0)
                nc.scalar.activation(r_t[:, lsl], ps_ri, AF.Sigmoid,
                                     bias=br1_sb[:, hsl], scale=1.0)
                nc.scalar.activation(omr_t[:, lsl], ps_ri, AF.Sigmoid,
                                     bias=nbr1_sb[:, hsl], scale=-1.0)
                # g = (1-f) * xt, in place over omf
                nc.vector.tensor_mul(omf_t[:, lsl], omf_t[:, lsl], ps_xt)
            # recurrence: c = f*c + g  (chained scan over L halves)
            nc.vector.tensor_tensor_scan(
                cs_t[:, 0:HALF], f_t[:, 0:HALF], omf_t[:, 0:HALF],
                0.0, OP.mult, OP.add)
            for half in range(1, NHALF):
                lsl = ds(half * HALF, HALF)
                nc.vector.tensor_tensor_scan(
                    cs_t[:, lsl], f_t[:, lsl], omf_t[:, lsl],
                    cs_t[:, half * HALF - 1:half * HALF], OP.mult, OP.add)
            # highway: h1 = r*cs + (1-r)*x
            nc.vector.tensor_mul(r_t, r_t, cs_t)
            nc.gpsimd.tensor_mul(omr_t, omr_t, xT[hc])
            h1_t = h1pool.tile([P, L], BF16, tag="h1", name="h1")
            nc.gpsimd.tensor_add(h1_t, r_t, omr_t)
            h1.append(h1_t)
        return h1

    # ---- SRU layer 2: highway only at last timestep -> last_bf columns ----
    def layer2(b, h1):
        for hc in range(HC):
            f_t = gates.tile([P, L], F32, tag="f", name="f")
            omf_t = gates.tile([P, L], F32, tag="omf", name="omf")
            cs_t = csp.tile([P, L], F32, tag="cs", name="cs")
            for half in range(NHALF):
                lsl = ds(half * HALF, HALF)
                ps_xt = ps_g.tile([P, HALF], F32, tag="ps_xt", name="ps_xt",
                                  bufs=3 if cfg.dma_transpose else 2)
                ps_fi = ps_g.tile([P, HALF], F32, tag="ps_fi", name="ps_fi")
                gate_matmuls(ps_xt, w2bf, 0, hc, h1, lsl)
                gate_matmuls(ps_fi, w2bf, 1, hc, h1, lsl)
                hsl = ds(hc, 1)
                nc.scalar.activation(f_t[:, lsl], ps_fi, AF.Sigmoid,
                                     bias=bf2_sb[:, hsl], scale=1.0)
                nc.scalar.activation(omf_t[:, lsl], ps_fi, AF.Sigmoid,
                                     bias=nbf2_sb[:, hsl], scale=-1.0)
                nc.vector.tensor_mul(omf_t[:, lsl], omf_t[:, lsl], ps_xt)
            nc.vector.tensor_tensor_scan(
                cs_t[:, 0:HALF], f_t[:, 0:HALF], omf_t[:, 0:HALF],
                0.0, OP.mult, OP.add)
            for half in range(1, NHALF):
                lsl = ds(half * HALF, HALF)
                nc.vector.tensor_tensor_scan(
                    cs_t[:, lsl], f_t[:, lsl], omf_t[:, lsl],
                    cs_t[:, half * HALF - 1:half * HALF], OP.mult, OP.add)
            # r2 only needed at t = L-1
            ps_ri2 = ps_misc.tile([P, 1], F32, tag="misc", name="misc")
            for kc in range(KC):
                nc.tensor.matmul(
                    ps_ri2, w2bf[kc][:, ds(2 * H + hc * P, P)],
                    h1[kc][:, L - 1:L], start=kc == 0, stop=kc == KC - 1)
            r2 = smalls.tile([P, 1], F32, tag="r2", name="r2")
            omr2 = smalls.tile([P, 1], F32, tag="omr2", name="omr2")
            hsl = ds(hc, 1)
            nc.scalar.activation(r2, ps_ri2, AF.Sigmoid,
                                 bias=br2_sb[:, hsl], scale=1.0)
            nc.scalar.activation(omr2, ps_ri2, AF.Sigmoid,
                                 bias=nbr2_sb[:, hsl], scale=-1.0)
            nc.vector.tensor_mul(r2, r2, cs_t[:, L - 1:L])
            nc.vector.tensor_mul(omr2, omr2, h1[hc][:, L - 1:L])
            nc.vector.tensor_add(last_bf[hc][:, ds(b, 1)], r2, omr2)

    for b in range(BL):
        xT = transpose_batch(b)
        h1 = layer1(b, xT)
        layer2(b, h1)

    # ---- FC head: out[b, c] = sum_h last[h, b] * Wfc[h, c] + bfc[c] ----
    out_sb = singles.tile([BL, C], F32, tag="out_sb", name="out_sb")
    chalf = C // 2
    for nh in range(2):
        csl = ds(nh * chalf, chalf)
        ps_fc = ps_misc.tile([BL, chalf], F32, tag="misc", name="misc")
        for kc in range(KC):
            nc.tensor.matmul(ps_fc, last_bf[kc], wfcbf[kc][:, csl],
                             start=kc == 0, stop=False)
        nc.tensor.matmul(ps_fc, ones_bf, bfc_bf[:, csl], start=False, stop=True)
        nc.scalar.copy(out_sb[:, csl], ps_fc)
    nc.sync.dma_start(out=out, in_=out_sb)


def build_program(cfg: Cfg = Cfg()):
    nc = bacc.Bacc("TRN2", target_bir_lowering=False, num_devices=cfg.n_cores)
    x = nc.dram_tensor("x", [cfg.BL, cfg.L, cfg.D], F32, kind="ExternalInput").ap()
    w1 = nc.dram_tensor("w1", [cfg.D, 3 * cfg.H], F32, kind="ExternalInput").ap()
    bf1 = nc.dram_tensor("bf1", [cfg.H], F32, kind="ExternalInput").ap()
    br1 = nc.dram_tensor("br1", [cfg.H], F32, kind="ExternalInput").ap()
    w2 = nc.dram_tensor("w2", [cfg.H, 3 * cfg.H], F32, kind="ExternalInput").ap()
    bf2 = nc.dram_tensor("bf2", [cfg.H], F32, kind="ExternalInput").ap()
    br2 = nc.dram_tensor("br2", [cfg.H], F32, kind="ExternalInput").ap()
    wfc = nc.dram_tensor("wfc", [cfg.H, cfg.C], F32, kind="ExternalInput").ap()
    bfc = nc.dram_tensor("bfc", [cfg.C], F32, kind="ExternalInput").ap()
    out = nc.dram_tensor("out", [cfg.BL, cfg.C], F32, kind="ExternalOutput").ap()
    aps = (x, w1, bf1, br1, w2, bf2, br2, wfc, bfc, out)
    with tile.TileContext(nc) as tc:
        with ExitStack() as ctx:
            _build_body(ctx, tc, cfg, aps)
    nc.compile()
    return nc


def make_in_maps(inputs, cfg: Cfg = Cfg()):
    """Shard the full inputs: batch-slice x per core, replicate weights."""
    x = np.ascontiguousarray(inputs["x"])  # [L, B, D]
    shared = {
        k: np.ascontiguousarray(np.asarray(inputs[k], dtype=np.float32))
        for k in ("W1", "bf1", "br1", "W2", "bf2", "br2", "Wfc", "bfc")
    }
    rename = {"W1": "w1", "W2": "w2", "Wfc": "wfc"}
    shared = {rename.get(k, k): v for k, v in shared.items()}
    in_maps = []
    for core in range(cfg.n_cores):
        bsl = slice(core * cfg.BL, (core + 1) * cfg.BL)
        # [L, BL, D] -> [BL, L, D] contiguous, best DMA layout
        x_loc = np.ascontiguousarray(
            x[:, bsl, :].transpose(1, 0, 2).astype(np.float32))
        in_maps.append({"x": x_loc, **shared})
    return in_maps


_CACHE = {}


def kernel(**inputs) -> np.ndarray:
    cfg = Cfg()
    if "nc" not in _CACHE:
        _CACHE["nc"] = build_program(cfg)
    nc = _CACHE["nc"]
    in_maps = make_in_maps(inputs, cfg)
    res = run_bass_kernel_spmd(nc, in_maps, core_ids=list(range(cfg.n_cores)))
    outs = [res.results[c]["out"] for c in range(cfg.n_cores)]
    return np.concatenate(outs, axis=0).astype(np.float32)


if __name__ == "__main__":
    rng = np.random.default_rng(0)
    cfg = Cfg()
    fake = {
        "x": rng.standard_normal((cfg.L, 8 * cfg.BL, cfg.D), dtype=np.float32),
        "W1": rng.standard_normal((cfg.D, 3 * cfg.H), dtype=np.float32) * 0.02,
        "bf1": np.zeros(cfg.H, np.float32),
        "br1": np.zeros(cfg.H, np.float32),
        "W2": rng.standard_normal((cfg.H, 3 * cfg.H), dtype=np.float32) * 0.02,
        "bf2": np.zeros(cfg.H, np.float32),
        "br2": np.zeros(cfg.H, np.float32),
        "Wfc": rng.standard_normal((cfg.H, cfg.C), dtype=np.float32) * 0.02,
        "bfc": np.zeros(cfg.C, np.float32),
    }
    out = kernel(**fake)
    print("kernel output", out.shape, out.dtype)


# revision 15
# speedup vs baseline: 1.7326x; 1.2434x over previous
"""SRU (Simple Recurrent Unit) 2-layer network + FC head on 8 Trainium2 cores.

Model (per reference):
  U = x @ W          -> xt, fi, ri gates            [L, B, 3H]
  f = sigmoid(fi+bf); r = sigmoid(ri+br)
  c_t = f_t * c_{t-1} + (1-f_t) * xt_t              (scan over L)
  h = r * c + (1-r) * x                             (highway)
  out = h2[last] @ Wfc + bfc

Sharding: data-parallel over batch (B=32 -> 4 per core), weights replicated.

Per-core kernel layout strategy: compute U^T per batch in [3H, L] layout
(W chunk stationary, x^T moving) so time lies along the SBUF free dim --
the hardware tensor_tensor_scan instruction then runs the recurrence
(state = f*state + g) 128 lanes at a time.  The scan/highway output [H, L]
is directly the moving operand of the next layer's GEMM, so only the input
x needs an on-chip transpose (PE transpose via identity).  Layer 2 only
needs its r gate at the last timestep (only h2[-1] feeds the FC head), so
its ri GEMM is a single-column matvec and the full highway is skipped.
"""

import os
import sys
import tempfile
from contextlib import ExitStack
from dataclasses import dataclass

import numpy as np

try:
    import concourse.bass as bass
except ImportError:
    sys.path.insert(0, "/opt/trn_rl_repo")
    import concourse.bass as bass

import concourse.mybir as mybir
import concourse.tile as tile
from concourse import bacc
from concourse.bass import ds, ts
from concourse.bass_utils import run_bass_kernel_spmd
from concourse.masks import make_identity

P = 128
F32 = mybir.dt.float32
BF16 = mybir.dt.bfloat16
AF = mybir.ActivationFunctionType
OP = mybir.AluOpType


@dataclass(frozen=True)
class Cfg:
    L: int = 1024          # timesteps
    BL: int = 4            # batch per core
    D: int = 768           # input dim (== H)
    H: int = 768           # hidden dim
    C: int = 1000          # classes
    n_cores: int = 8
    # x^T via whole-column DRAM->SBUF xbar-transpose DMAs (24 ops).  The
    # SBUF->SBUF [128,128]-block variant serializes the sync engine
    # (~1.2us/trigger x 192) and regressed to 700us -- large DRAM-source
    # transposes amortize the trigger cost.
    dma_transpose: bool = True

    @property
    def KC(self):          # contraction chunks of 128
        return self.D // P

    @property
    def HC(self):          # hidden chunks of 128
        return self.H // P

    @property
    def LB(self):          # timestep blocks of 128
        return self.L // P

    @property
    def HALF(self):        # PSUM-bank-sized slice of L
        return min(self.L, 512)

    @property
    def NHALF(self):
        return self.L // self.HALF


def _build_body(ctx, tc, cfg, aps):
    nc = tc.nc
    L, BL, D, H, C = cfg.L, cfg.BL, cfg.D, cfg.H, cfg.C
    KC, HC, LB, HALF, NHALF = cfg.KC, cfg.HC, cfg.LB, cfg.HALF, cfg.NHALF
    x, w1, bf1, br1, w2, bf2, br2, wfc, bfc, out = aps

    singles = ctx.enter_context(tc.tile_pool(name="singles", bufs=1))
    xTpool = ctx.enter_context(tc.tile_pool(name="xT", bufs=2 * KC))
    h1pool = ctx.enter_context(tc.tile_pool(name="h1", bufs=2 * KC))
    gates = ctx.enter_context(tc.tile_pool(name="gates", bufs=2))
    csp = ctx.enter_context(tc.tile_pool(name="cs", bufs=2))
    smalls = ctx.enter_context(tc.tile_pool(name="smalls", bufs=4))
    ps_g = ctx.enter_context(tc.tile_pool(name="ps_g", bufs=2, space="PSUM"))
    if not cfg.dma_transpose:
        xstage = ctx.enter_context(tc.tile_pool(name="xstage", bufs=LB))
        ps_tp = ctx.enter_context(tc.tile_pool(name="ps_tp", bufs=1, space="PSUM"))
        ident = singles.tile([P, P], BF16, tag="ident", name="ident")
        make_identity(nc, ident)

    ps_misc = ctx.enter_context(tc.tile_pool(name="ps_misc", bufs=1, space="PSUM"))

    # ---- weights arrive bf16 from the host shard step: straight DMA loads,
    # column-halves so the first gate groups unblock early ----
    def load_weight(wap, ncols, nm):
        tiles = []
        for kc in range(KC):
            wt = singles.tile([P, ncols], BF16, tag=f"{nm}_{kc}", name=f"{nm}_{kc}")
            tiles.append(wt)
        for ch in range(2):
            half = ncols // 2
            csl = ds(ch * half, half)
            for kc in range(KC):
                nc.sync.dma_start(out=tiles[kc][:, csl], in_=wap[ts(kc, P), csl])
        return tiles

    w1bf = load_weight(w1, 3 * H, "w1")
    w2bf = load_weight(w2, 3 * H, "w2")
    wfcbf = load_weight(wfc, C, "wfc")

    # ---- biases as [P, HC] column tiles (+ negated for 1-sigmoid trick) ----
    def load_bias(vec, nm):
        t = singles.tile([P, HC], F32, tag=f"b_{nm}", name=f"b_{nm}")
        nt = singles.tile([P, HC], F32, tag=f"nb_{nm}", name=f"nb_{nm}")
        nc.sync.dma_start(out=t, in_=vec.rearrange("(c p) -> p c", p=P))
        nc.scalar.mul(nt, t, -1.0)
        return t, nt

    bf1_sb, nbf1_sb = load_bias(bf1, "f1")
    br1_sb, nbr1_sb = load_bias(br1, "r1")
    bf2_sb, nbf2_sb = load_bias(bf2, "f2")
    br2_sb, nbr2_sb = load_bias(br2, "r2")

    bfc_f32 = singles.tile([1, C], F32, tag="bfc32", name="bfc32")
    nc.sync.dma_start(out=bfc_f32, in_=bfc[None, :])
    bfc_bf = singles.tile([1, C], BF16, tag="bfc16", name="bfc16")
    nc.gpsimd.tensor_copy(bfc_bf, bfc_f32)
    ones_bf = singles.tile([1, BL], BF16, tag="ones", name="ones")
    nc.vector.memset(ones_bf, 1.0)

    last_bf = [singles.tile([P, BL], BF16, tag=f"last_{hc}", name=f"last_{hc}") for hc in range(HC)]

    # ---- x[b] (bf16 [L, D] in DRAM) -> x^T [D, L] in SBUF ----
    def transpose_batch(b):
        xT = []
        if cfg.dma_transpose:
            # one whole-column xbar-transpose DMA per 128-wide d-chunk
            for dc in range(KC):
                xt_t = xTpool.tile([P, L], BF16, tag="xT", name="xT")
                nc.sync.dma_start_transpose(out=xt_t, in_=x[b, :, ts(dc, P)])
                xT.append(xt_t)
            return xT
        xs = []
        for lb in range(LB):
            t = xstage.tile([P, D], BF16, tag="xs", name="xs")
            nc.sync.dma_start(out=t, in_=x[b, ts(lb, P), :])
            xs.append(t)
        for dc in range(KC):
            xt_t = xTpool.tile([P, L], BF16, tag="xT", name="xT")
            for half in range(NHALF):
                pt = ps_tp.tile([P, HALF], BF16, tag="tp", name="tp")
                for j in range(HALF // P):
                    lb = half * (HALF // P) + j
                    nc.tensor.transpose(pt[:, ts(j, P)], xs[lb][:, ts(dc, P)], ident)
                nc.scalar.copy(xt_t[:, ds(half * HALF, HALF)], pt)
            xT.append(xt_t)
        return xT

    def gate_matmuls(ps, wbf, gate, hc, rhs_tiles, lsl):
        for kc in range(KC):
            nc.tensor.matmul(
                ps,
                wbf[kc][:, ds(gate * H + hc * P, P)],
                rhs_tiles[kc][:, lsl],
                start=kc == 0,
                stop=kc == KC - 1,
            )

    # ---- SRU layer 1: full highway, h1 in [H, L] bf16 ----
    def layer1(b, xT):
        h1 = []
        for hc in range(HC):
            f_t = gates.tile([P, L], F32, tag="f", name="f")
            omf_t = gates.tile([P, L], F32, tag="omf", name="omf")
            r_t = gates.tile([P, L], F32, tag="r", name="r")
            omr_t = gates.tile([P, L], F32, tag="omr", name="omr")
            cs_t = csp.tile([P, L], F32, tag="cs", name="cs")
            for half in range(NHALF):
                lsl = ds(half * HALF, HALF)
                ps_xt = ps_g.tile([P, HALF], F32, tag="ps_xt", name="ps_xt",
                                  bufs=3 if cfg.dma_transpose else 2)
                ps_fi = ps_g.tile([P, HALF], F32, tag="ps_fi", name="ps_fi")
                ps_ri = ps_g.tile([P, HALF], F32, tag="ps_ri", name="ps_ri")
                gate_matmuls(ps_xt, w1bf, 0, hc, xT, lsl)
                gate_matmuls(ps_fi, w1bf, 1, hc, xT, lsl)
                gate_matmuls(ps_ri, w1bf, 2, hc, xT, lsl)
                hsl = ds(hc, 1)
                nc.scalar.activation(f_t[:, lsl], ps_fi, AF.Sigmoid,
                                     bias=bf1_sb[:, hsl], scale=1.0)
                nc.scalar.activation(omf_t[:, lsl], ps_fi, AF.Sigmoid,
                                     bias=nbf1_sb[:, hsl], scale=-1.0)
                nc.scalar.activation(r_t[:, lsl], ps_ri, AF.Sigmoid,
                                     bias=br1_sb[:, hsl], scale=1.0)
                nc.scalar.activation(omr_t[:, lsl], ps_ri, AF.Sigmoid,
                                     bias=nbr1_sb[:, hsl], scale=-1.0)
                # g = (1-f) * xt, in place over omf
                nc.vector.tensor_mul(omf_t[:, lsl], omf_t[:, lsl], ps_xt)
            # recurrence: c = f*c + g  (chained scan over L halves)
            nc.vector.tensor_tensor_scan(
                cs_t[:, 0:HALF], f_t[:, 0:HALF], omf_t[:, 0:HALF],
                0.0, OP.mult, OP.add)
            for half in range(1, NHALF):
                lsl = ds(half * HALF, HALF)
                nc.vector.tensor_tensor_scan(
                    cs_t[:, lsl], f_t[:, lsl], omf_t[:, lsl],
                    cs_t[:, half * HALF - 1:half * HALF], OP.mult, OP.add)
            # highway: h1 = r*cs + (1-r)*x
            nc.vector.tensor_mul(r_t, r_t, cs_t)
            nc.gpsimd.tensor_mul(omr_t, omr_t, xT[hc])
            h1_t = h1pool.tile([P, L], BF16, tag="h1", name="h1")
            nc.gpsimd.tensor_add(h1_t, r_t, omr_t)
            h1.append(h1_t)
        return h1

    # ---- SRU layer 2: highway only at last timestep -> last_bf columns ----
    def layer2(b, h1):
        for hc in range(HC):
            f_t = gates.tile([P, L], F32, tag="f", name="f")
            omf_t = gates.tile([P, L], F32, tag="omf", name="omf")
            cs_t = csp.tile([P, L], F32, tag="cs", name="cs")
            for half in range(NHALF):
                lsl = ds(half * HALF, HALF)
                ps_xt = ps_g.tile([P, HALF], F32, tag="ps_xt", name="ps_xt",
                                  bufs=3 if cfg.dma_transpose else 2)
                ps_fi = ps_g.tile([P, HALF], F32, tag="ps_fi", name="ps_fi")
                gate_matmuls(ps_xt, w2bf, 0, hc, h1, lsl)
                gate_matmuls(ps_fi, w2bf, 1, hc, h1, lsl)
                hsl = ds(hc, 1)
                nc.scalar.activation(f_t[:, lsl], ps_fi, AF.Sigmoid,
                                     bias=bf2_sb[:, hsl], scale=1.0)
                nc.scalar.activation(omf_t[:, lsl], ps_fi, AF.Sigmoid,
                                     bias=nbf2_sb[:, hsl], scale=-1.0)
                nc.vector.tensor_mul(omf_t[:, lsl], omf_t[:, lsl], ps_xt)
            nc.vector.tensor_tensor_scan(
                cs_t[:, 0:HALF], f_t[:, 0:HALF], omf_t[:, 0:HALF],
                0.0, OP.mult, OP.add)
            for half in range(1, NHALF):
                lsl = ds(half * HALF, HALF)
                nc.vector.tensor_tensor_scan(
                    cs_t[:, lsl], f_t[:, lsl], omf_t[:, lsl],
                    cs_t[:, half * HALF - 1:half * HALF], OP.mult, OP.add)
            # r2 only needed at t = L-1
            ps_ri2 = ps_misc.tile([P, 1], F32, tag="misc", name="misc")
            for kc in range(KC):
                nc.tensor.matmul(
                    ps_ri2, w2bf[kc][:, ds(2 * H + hc * P, P)],
                    h1[kc][:, L - 1:L], start=kc == 0, stop=kc == KC - 1)
            r2 = smalls.tile([P, 1], F32, tag="r2", name="r2")
            omr2 = smalls.tile([P, 1], F32, tag="omr2", name="omr2")
            hsl = ds(hc, 1)
            nc.scalar.activation(r2, ps_ri2, AF.Sigmoid,
                                 bias=br2_sb[:, hsl], scale=1.0)
            nc.scalar.activation(omr2, ps_ri2, AF.Sigmoid,
                                 bias=nbr2_sb[:, hsl], scale=-1.0)
            nc.vector.tensor_mul(r2, r2, cs_t[:, L - 1:L])
            nc.vector.tensor_mul(omr2, omr2, h1[hc][:, L - 1:L])
            nc.vector.tensor_add(last_bf[hc][:, ds(b, 1)], r2, omr2)

    for b in range(BL):
        xT = transpose_batch(b)
        h1 = layer1(b, xT)
        layer2(b, h1)

    # ---- FC head: out[b, c] = sum_h last[h, b] * Wfc[h, c] + bfc[c] ----
    out_sb = singles.tile([BL, C], F32, tag="out_sb", name="out_sb")
    chalf = C // 2
    for nh in range(2):
        csl = ds(nh * chalf, chalf)
        ps_fc = ps_misc.tile([BL, chalf], F32, tag="misc", name="misc")
        for kc in range(KC):
            nc.tensor.matmul(ps_fc, last_bf[kc], wfcbf[kc][:, csl],
                             start=kc == 0, stop=False)
        nc.tensor.matmul(ps_fc, ones_bf, bfc_bf[:, csl], start=False, stop=True)
        nc.scalar.copy(out_sb[:, csl], ps_fc)
    nc.sync.dma_start(out=out, in_=out_sb)


def build_program(cfg: Cfg = Cfg()):
    nc = bacc.Bacc("TRN2", target_bir_lowering=False, num_devices=cfg.n_cores)
    x = nc.dram_tensor("x", [cfg.BL, cfg.L, cfg.D], BF16, kind="ExternalInput").ap()
    w1 = nc.dram_tensor("w1", [cfg.D, 3 * cfg.H], BF16, kind="ExternalInput").ap()
    bf1 = nc.dram_tensor("bf1", [cfg.H], F32, kind="ExternalInput").ap()
    br1 = nc.dram_tensor("br1", [cfg.H], F32, kind="ExternalInput").ap()
    w2 = nc.dram_tensor("w2", [cfg.H, 3 * cfg.H], BF16, kind="ExternalInput").ap()
    bf2 = nc.dram_tensor("bf2", [cfg.H], F32, kind="ExternalInput").ap()
    br2 = nc.dram_tensor("br2", [cfg.H], F32, kind="ExternalInput").ap()
    wfc = nc.dram_tensor("wfc", [cfg.H, cfg.C], BF16, kind="ExternalInput").ap()
    bfc = nc.dram_tensor("bfc", [cfg.C], F32, kind="ExternalInput").ap()
    out = nc.dram_tensor("out", [cfg.BL, cfg.C], F32, kind="ExternalOutput").ap()
    aps = (x, w1, bf1, br1, w2, bf2, br2, wfc, bfc, out)
    with tile.TileContext(nc) as tc:
        with ExitStack() as ctx:
            _build_body(ctx, tc, cfg, aps)
    nc.compile()
    return nc


def make_in_maps(inputs, cfg: Cfg = Cfg()):
    """Shard the full inputs: batch-slice x per core, replicate weights.

    x and the weight matrices are staged to bf16 host-side (the kernel's
    matmuls consume bf16 anyway); biases stay f32.
    """
    import ml_dtypes
    bf = ml_dtypes.bfloat16
    x = np.asarray(inputs["x"], dtype=np.float32)  # [L, B, D]
    shared = {}
    for k in ("W1", "bf1", "br1", "W2", "bf2", "br2", "Wfc", "bfc"):
        v = np.asarray(inputs[k], dtype=np.float32)
        if k in ("W1", "W2", "Wfc"):
            v = v.astype(bf)
        shared[k.lower() if k in ("W1", "W2", "Wfc") else k] = (
            np.ascontiguousarray(v))
    in_maps = []
    for core in range(cfg.n_cores):
        bsl = slice(core * cfg.BL, (core + 1) * cfg.BL)
        # [L, BL, D] -> [BL, L, D] contiguous bf16
        x_loc = np.ascontiguousarray(
            x[:, bsl, :].transpose(1, 0, 2).astype(bf))
        in_maps.append({"x": x_loc, **shared})
    return in_maps


_CACHE = {}


def kernel(**inputs) -> np.ndarray:
    cfg = Cfg()
    if "nc" not in _CACHE:
        _CACHE["nc"] = build_program(cfg)
    nc = _CACHE["nc"]
    in_maps = make_in_maps(inputs, cfg)
    res = run_bass_kernel_spmd(nc, in_maps, core_ids=list(range(cfg.n_cores)))
    outs = [res.results[c]["out"] for c in range(cfg.n_cores)]
    return np.concatenate(outs, axis=0).astype(np.float32)


if __name__ == "__main__":
    rng = np.random.default_rng(0)
    cfg = Cfg()
    fake = {
        "x": rng.standard_normal((cfg.L, 8 * cfg.BL, cfg.D), dtype=np.float32),
        "W1": rng.standard_normal((cfg.D, 3 * cfg.H), dtype=np.float32) * 0.02,
        "bf1": np.zeros(cfg.H, np.float32),
        "br1": np.zeros(cfg.H, np.float32),
        "W2": rng.standard_normal((cfg.H, 3 * cfg.H), dtype=np.float32) * 0.02,
        "bf2": np.zeros(cfg.H, np.float32),
        "br2": np.zeros(cfg.H, np.float32),
        "Wfc": rng.standard_normal((cfg.H, cfg.C), dtype=np.float32) * 0.02,
        "bfc": np.zeros(cfg.C, np.float32),
    }
    out = kernel(**fake)
    print("kernel output", out.shape, out.dtype)


# revision 18
# speedup vs baseline: 1.7521x; 1.0113x over previous
"""SRU (Simple Recurrent Unit) 2-layer network + FC head on 8 Trainium2 cores.

Model (per reference):
  U = x @ W          -> xt, fi, ri gates            [L, B, 3H]
  f = sigmoid(fi+bf); r = sigmoid(ri+br)
  c_t = f_t * c_{t-1} + (1-f_t) * xt_t              (scan over L)
  h = r * c + (1-r) * x                             (highway)
  out = h2[last] @ Wfc + bfc

Sharding: data-parallel over batch (B=32 -> 4 per core), weights replicated.

Per-core kernel layout strategy: compute U^T per batch in [3H, L] layout
(W chunk stationary, x^T moving) so time lies along the SBUF free dim --
the hardware tensor_tensor_scan instruction then runs the recurrence
(state = f*state + g) 128 lanes at a time.  The scan/highway output [H, L]
is directly the moving operand of the next layer's GEMM, so only the input
x needs an on-chip transpose (PE transpose via identity).  Layer 2 only
needs its r gate at the last timestep (only h2[-1] feeds the FC head), so
its ri GEMM is a single-column matvec and the full highway is skipped.
"""

import os
import sys
import tempfile
from contextlib import ExitStack
from dataclasses import dataclass

import numpy as np

try:
    import concourse.bass as bass
except ImportError:
    sys.path.insert(0, "/opt/trn_rl_repo")
    import concourse.bass as bass

import concourse.mybir as mybir
import concourse.tile as tile
from concourse import bacc
from concourse.bass import ds, ts
from concourse.bass_utils import run_bass_kernel_spmd
from concourse.masks import make_identity

P = 128
F32 = mybir.dt.float32
BF16 = mybir.dt.bfloat16
AF = mybir.ActivationFunctionType
OP = mybir.AluOpType


@dataclass(frozen=True)
class Cfg:
    L: int = 1024          # timesteps
    BL: int = 4            # batch per core
    D: int = 768           # input dim (== H)
    H: int = 768           # hidden dim
    C: int = 1000          # classes
    n_cores: int = 8
    # x^T via whole-column DRAM->SBUF xbar-transpose DMAs (24 ops).  The
    # SBUF->SBUF [128,128]-block variant serializes the sync engine
    # (~1.2us/trigger x 192) and regressed to 700us -- large DRAM-source
    # transposes amortize the trigger cost.
    dma_transpose: bool = True

    @property
    def KC(self):          # contraction chunks of 128
        return self.D // P

    @property
    def HC(self):          # hidden chunks of 128
        return self.H // P

    @property
    def LB(self):          # timestep blocks of 128
        return self.L // P

    @property
    def HALF(self):        # PSUM-bank-sized slice of L
        return min(self.L, 512)

    @property
    def NHALF(self):
        return self.L // self.HALF


def _build_body(ctx, tc, cfg, aps):
    nc = tc.nc
    L, BL, D, H, C = cfg.L, cfg.BL, cfg.D, cfg.H, cfg.C
    KC, HC, LB, HALF, NHALF = cfg.KC, cfg.HC, cfg.LB, cfg.HALF, cfg.NHALF
    x, w1, bf1, br1, w2, bf2, br2, wfc, bfc, out = aps

    singles = ctx.enter_context(tc.tile_pool(name="singles", bufs=1))
    xTpool = ctx.enter_context(tc.tile_pool(name="xT", bufs=2 * KC))
    h1pool = ctx.enter_context(tc.tile_pool(name="h1", bufs=2 * KC))
    gates = ctx.enter_context(tc.tile_pool(name="gates", bufs=3))
    csp = ctx.enter_context(tc.tile_pool(name="cs", bufs=3))
    smalls = ctx.enter_context(tc.tile_pool(name="smalls", bufs=4))
    ps_g = ctx.enter_context(tc.tile_pool(name="ps_g", bufs=2, space="PSUM"))
    if not cfg.dma_transpose:
        xstage = ctx.enter_context(tc.tile_pool(name="xstage", bufs=LB))
        ps_tp = ctx.enter_context(tc.tile_pool(name="ps_tp", bufs=1, space="PSUM"))
        ident = singles.tile([P, P], BF16, tag="ident", name="ident")
        make_identity(nc, ident)

    # ---- weights arrive bf16 from the host shard step: straight DMA loads,
    # column-halves so the first gate groups unblock early ----
    def load_weight(wap, ncols, nm):
        tiles = []
        for kc in range(KC):
            wt = singles.tile([P, ncols], BF16, tag=f"{nm}_{kc}", name=f"{nm}_{kc}")
            tiles.append(wt)
        for ch in range(2):
            half = ncols // 2
            csl = ds(ch * half, half)
            for kc in range(KC):
                nc.sync.dma_start(out=tiles[kc][:, csl], in_=wap[ts(kc, P), csl])
        return tiles

    # ---- biases as [P, HC] column tiles (+ negated for 1-sigmoid trick) ----
    def load_bias(vec, nm):
        t = singles.tile([P, HC], F32, tag=f"b_{nm}", name=f"b_{nm}")
        nt = singles.tile([P, HC], F32, tag=f"nb_{nm}", name=f"nb_{nm}")
        nc.sync.dma_start(out=t, in_=vec.rearrange("(c p) -> p c", p=P))
        nc.scalar.mul(nt, t, -1.0)
        return t, nt

    bf1_sb, nbf1_sb = load_bias(bf1, "f1")
    br1_sb, nbr1_sb = load_bias(br1, "r1")
    bf2_sb, nbf2_sb = load_bias(bf2, "f2")
    br2_sb, nbr2_sb = load_bias(br2, "r2")

    bfc_f32 = singles.tile([1, C], F32, tag="bfc32", name="bfc32")
    nc.sync.dma_start(out=bfc_f32, in_=bfc[None, :])
    bfc_bf = singles.tile([1, C], BF16, tag="bfc16", name="bfc16")
    nc.gpsimd.tensor_copy(bfc_bf, bfc_f32)
    ones_bf = singles.tile([1, BL], BF16, tag="ones", name="ones")
    nc.vector.memset(ones_bf, 1.0)

    last_bf = [singles.tile([P, BL], BF16, tag=f"last_{hc}", name=f"last_{hc}") for hc in range(HC)]

    # ---- x[b] (bf16 [L, D] in DRAM) -> x^T [D, L] in SBUF ----
    def transpose_batch(b):
        xT = []
        if cfg.dma_transpose:
            # one whole-column xbar-transpose DMA per 128-wide d-chunk
            for dc in range(KC):
                xt_t = xTpool.tile([P, L], BF16, tag="xT", name="xT")
                nc.sync.dma_start_transpose(out=xt_t, in_=x[b, :, ts(dc, P)])
                xT.append(xt_t)
            return xT
        xs = []
        for lb in range(LB):
            t = xstage.tile([P, D], BF16, tag="xs", name="xs")
            nc.sync.dma_start(out=t, in_=x[b, ts(lb, P), :])
            xs.append(t)
        for dc in range(KC):
            xt_t = xTpool.tile([P, L], BF16, tag="xT", name="xT")
            for half in range(NHALF):
                pt = ps_tp.tile([P, HALF], BF16, tag="tp", name="tp")
                for j in range(HALF // P):
                    lb = half * (HALF // P) + j
                    nc.tensor.transpose(pt[:, ts(j, P)], xs[lb][:, ts(dc, P)], ident)
                nc.scalar.copy(xt_t[:, ds(half * HALF, HALF)], pt)
            xT.append(xt_t)
        return xT

    def gate_matmuls(ps, wbf, gate, hc, rhs_tiles, lsl):
        for kc in range(KC):
            nc.tensor.matmul(
                ps,
                wbf[kc][:, ds(gate * H + hc * P, P)],
                rhs_tiles[kc][:, lsl],
                start=kc == 0,
                stop=kc == KC - 1,
            )

    # ---- SRU layer 1: full highway, h1 in [H, L] bf16 ----
    def layer1(b, xT):
        h1 = []
        for hc in range(HC):
            f_t = gates.tile([P, L], BF16, tag="f", name="f")
            omf_t = gates.tile([P, L], BF16, tag="omf", name="omf")
            r_t = gates.tile([P, L], BF16, tag="r", name="r")
            omr_t = gates.tile([P, L], BF16, tag="omr", name="omr")
            cs_t = csp.tile([P, L], BF16, tag="cs", name="cs")
            for half in range(NHALF):
                lsl = ds(half * HALF, HALF)
                ps_xt = ps_g.tile([P, HALF], F32, tag="ps_xt", name="ps_xt",
                                  bufs=3 if cfg.dma_transpose else 2)
                ps_fi = ps_g.tile([P, HALF], F32, tag="ps_fi", name="ps_fi",
                                  bufs=3)
                ps_ri = ps_g.tile([P, HALF], F32, tag="ps_ri", name="ps_ri")
                gate_matmuls(ps_xt, w1bf, 0, hc, xT, lsl)
                gate_matmuls(ps_fi, w1bf, 1, hc, xT, lsl)
                gate_matmuls(ps_ri, w1bf, 2, hc, xT, lsl)
                hsl = ds(hc, 1)
                nc.scalar.activation(f_t[:, lsl], ps_fi, AF.Sigmoid,
                                     bias=bf1_sb[:, hsl], scale=1.0)
                nc.scalar.activation(omf_t[:, lsl], ps_fi, AF.Sigmoid,
                                     bias=nbf1_sb[:, hsl], scale=-1.0)
                nc.scalar.activation(r_t[:, lsl], ps_ri, AF.Sigmoid,
                                     bias=br1_sb[:, hsl], scale=1.0)
                nc.scalar.activation(omr_t[:, lsl], ps_ri, AF.Sigmoid,
                                     bias=nbr1_sb[:, hsl], scale=-1.0)
                # g = (1-f) * xt, in place over omf
                nc.vector.tensor_mul(omf_t[:, lsl], omf_t[:, lsl], ps_xt)
            # recurrence: c = f*c + g  (chained scan over L halves)
            nc.vector.tensor_tensor_scan(
                cs_t[:, 0:HALF], f_t[:, 0:HALF], omf_t[:, 0:HALF],
                0.0, OP.mult, OP.add)
            for half in range(1, NHALF):
                lsl = ds(half * HALF, HALF)
                nc.vector.tensor_tensor_scan(
                    cs_t[:, lsl], f_t[:, lsl], omf_t[:, lsl],
                    cs_t[:, half * HALF - 1:half * HALF], OP.mult, OP.add)
            # highway: h1 = r*cs + (1-r)*x
            nc.vector.tensor_mul(r_t, r_t, cs_t)
            nc.gpsimd.tensor_mul(omr_t, omr_t, xT[hc])
            h1_t = h1pool.tile([P, L], BF16, tag="h1", name="h1")
            nc.gpsimd.tensor_add(h1_t, r_t, omr_t)
            h1.append(h1_t)
        return h1

    # per-hc last-timestep columns gathered across batches (for the batched
    # r2 gate + FC head at the end)
    cs2_last = [singles.tile([P, BL], BF16, tag=f"cs2l_{hc}", name=f"cs2l_{hc}")
                for hc in range(HC)]
    h1_last = [singles.tile([P, BL], BF16, tag=f"h1l_{hc}", name=f"h1l_{hc}")
               for hc in range(HC)]

    # ---- SRU layer 2: only cs2/h1 last-timestep columns are kept ----
    def layer2(b, h1):
        for hc in range(HC):
            f_t = gates.tile([P, L], BF16, tag="f", name="f")
            omf_t = gates.tile([P, L], BF16, tag="omf", name="omf")
            cs_t = csp.tile([P, L], BF16, tag="cs", name="cs")
            for half in range(NHALF):
                lsl = ds(half * HALF, HALF)
                ps_xt = ps_g.tile([P, HALF], F32, tag="ps_xt", name="ps_xt",
                                  bufs=3 if cfg.dma_transpose else 2)
                ps_fi = ps_g.tile([P, HALF], F32, tag="ps_fi", name="ps_fi",
                                  bufs=3)
                gate_matmuls(ps_xt, w2bf, 0, hc, h1, lsl)
                gate_matmuls(ps_fi, w2bf, 1, hc, h1, lsl)
                hsl = ds(hc, 1)
                nc.scalar.activation(f_t[:, lsl], ps_fi, AF.Sigmoid,
                                     bias=bf2_sb[:, hsl], scale=1.0)
                nc.scalar.activation(omf_t[:, lsl], ps_fi, AF.Sigmoid,
                                     bias=nbf2_sb[:, hsl], scale=-1.0)
                nc.vector.tensor_mul(omf_t[:, lsl], omf_t[:, lsl], ps_xt)
            nc.vector.tensor_tensor_scan(
                cs_t[:, 0:HALF], f_t[:, 0:HALF], omf_t[:, 0:HALF],
                0.0, OP.mult, OP.add)
            for half in range(1, NHALF):
                lsl = ds(half * HALF, HALF)
                nc.vector.tensor_tensor_scan(
                    cs_t[:, lsl], f_t[:, lsl], omf_t[:, lsl],
                    cs_t[:, half * HALF - 1:half * HALF], OP.mult, OP.add)
            nc.scalar.copy(cs2_last[hc][:, ds(b, 1)], cs_t[:, L - 1:L])
            nc.scalar.copy(h1_last[hc][:, ds(b, 1)], h1[hc][:, L - 1:L])

    xT_next = transpose_batch(0)  # b0 x^T DMAs queue ahead of the bulk W DMAs
    w1bf = load_weight(w1, 3 * H, "w1")
    w2bf = load_weight(w2, 3 * H, "w2")
    wfcbf = load_weight(wfc, C, "wfc")
    for b in range(BL):
        xT = xT_next
        h1 = layer1(b, xT)
        if b + 1 < BL:
            xT_next = transpose_batch(b + 1)
        layer2(b, h1)

    # ---- batched last-step r2 gate: last = r2*cs2 + (1-r2)*h1, all batches --
    for hc in range(HC):
        ps_ri2 = ps_g.tile([P, HALF], F32, tag="ps_fi", name="ps_ri2", bufs=3)
        for kc in range(KC):
            nc.tensor.matmul(
                ps_ri2[:, :BL], w2bf[kc][:, ds(2 * H + hc * P, P)],
                h1_last[kc], start=kc == 0, stop=kc == KC - 1)
        r2 = smalls.tile([P, BL], F32, tag="r2", name="r2")
        omr2 = smalls.tile([P, BL], F32, tag="omr2", name="omr2")
        hsl = ds(hc, 1)
        nc.scalar.activation(r2, ps_ri2[:, :BL], AF.Sigmoid,
                             bias=br2_sb[:, hsl], scale=1.0)
        nc.scalar.activation(omr2, ps_ri2[:, :BL], AF.Sigmoid,
                             bias=nbr2_sb[:, hsl], scale=-1.0)
        nc.vector.tensor_mul(r2, r2, cs2_last[hc])
        nc.vector.tensor_mul(omr2, omr2, h1_last[hc])
        nc.vector.tensor_add(last_bf[hc], r2, omr2)

    # ---- FC head: out[b, c] = sum_h last[h, b] * Wfc[h, c] + bfc[c] ----
    out_sb = singles.tile([BL, C], F32, tag="out_sb", name="out_sb")
    chalf = C // 2
    for nh in range(2):
        csl = ds(nh * chalf, chalf)
        ps_fc = ps_g.tile([P, HALF], F32, tag="ps_fi", name="ps_fc", bufs=3)
        for kc in range(KC):
            nc.tensor.matmul(ps_fc[:BL, :chalf], last_bf[kc], wfcbf[kc][:, csl],
                             start=kc == 0, stop=False)
        nc.tensor.matmul(ps_fc[:BL, :chalf], ones_bf, bfc_bf[:, csl],
                         start=False, stop=True)
        nc.scalar.copy(out_sb[:, csl], ps_fc[:BL, :chalf])
    nc.sync.dma_start(out=out, in_=out_sb)


def build_program(cfg: Cfg = Cfg()):
    nc = bacc.Bacc("TRN2", target_bir_lowering=False, num_devices=cfg.n_cores)
    x = nc.dram_tensor("x", [cfg.BL, cfg.L, cfg.D], BF16, kind="ExternalInput").ap()
    w1 = nc.dram_tensor("w1", [cfg.D, 3 * cfg.H], BF16, kind="ExternalInput").ap()
    bf1 = nc.dram_tensor("bf1", [cfg.H], F32, kind="ExternalInput").ap()
    br1 = nc.dram_tensor("br1", [cfg.H], F32, kind="ExternalInput").ap()
    w2 = nc.dram_tensor("w2", [cfg.H, 3 * cfg.H], BF16, kind="ExternalInput").ap()
    bf2 = nc.dram_tensor("bf2", [cfg.H], F32, kind="ExternalInput").ap()
    br2 = nc.dram_tensor("br2", [cfg.H], F32, kind="ExternalInput").ap()
    wfc = nc.dram_tensor("wfc", [cfg.H, cfg.C], BF16, kind="ExternalInput").ap()
    bfc = nc.dram_tensor("bfc", [cfg.C], F32, kind="ExternalInput").ap()
    out = nc.dram_tensor("out", [cfg.BL, cfg.C], F32, kind="ExternalOutput").ap()
    aps = (x, w1, bf1, br1, w2, bf2, br2, wfc, bfc, out)
    with tile.TileContext(nc) as tc:
        with ExitStack() as ctx:
            _build_body(ctx, tc, cfg, aps)
    nc.compile()
    return nc


def make_in_maps(inputs, cfg: Cfg = Cfg()):
    """Shard the full inputs: batch-slice x per core, replicate weights.

    x and the weight matrices are staged to bf16 host-side (the kernel's
    matmuls consume bf16 anyway); biases stay f32.
    """
    import ml_dtypes
    bf = ml_dtypes.bfloat16
    x = np.asarray(inputs["x"], dtype=np.float32)  # [L, B, D]
    shared = {}
    for k in ("W1", "bf1", "br1", "W2", "bf2", "br2", "Wfc", "bfc"):
        v = np.asarray(inputs[k], dtype=np.float32)
        if k in ("W1", "W2", "Wfc"):
            v = v.astype(bf)
        shared[k.lower() if k in ("W1", "W2", "Wfc") else k] = (
            np.ascontiguousarray(v))
    in_maps = []
    for core in range(cfg.n_cores):
        bsl = slice(core * cfg.BL, (core + 1) * cfg.BL)
        # [L, BL, D] -> [BL, L, D] contiguous bf16
        x_loc = np.ascontiguousarray(
            x[:, bsl, :].transpose(1, 0, 2).astype(bf))
        in_maps.append({"x": x_loc, **shared})
    return in_maps


_CACHE = {}


def kernel(**inputs) -> np.ndarray:
    cfg = Cfg()
    if "nc" not in _CACHE:
        _CACHE["nc"] = build_program(cfg)
    nc = _CACHE["nc"]
    in_maps = make_in_maps(inputs, cfg)
    res = run_bass_kernel_spmd(nc, in_maps, core_ids=list(range(cfg.n_cores)))
    outs = [res.results[c]["out"] for c in range(cfg.n_cores)]
    return np.concatenate(outs, axis=0).astype(np.float32)


if __name__ == "__main__":
    rng = np.random.default_rng(0)
    cfg = Cfg()
    fake = {
        "x": rng.standard_normal((cfg.L, 8 * cfg.BL, cfg.D), dtype=np.float32),
        "W1": rng.standard_normal((cfg.D, 3 * cfg.H), dtype=np.float32) * 0.02,
        "bf1": np.zeros(cfg.H, np.float32),
        "br1": np.zeros(cfg.H, np.float32),
        "W2": rng.standard_normal((cfg.H, 3 * cfg.H), dtype=np.float32) * 0.02,
        "bf2": np.zeros(cfg.H, np.float32),
        "br2": np.zeros(cfg.H, np.float32),
        "Wfc": rng.standard_normal((cfg.H, cfg.C), dtype=np.float32) * 0.02,
        "bfc": np.zeros(cfg.C, np.float32),
    }
    out = kernel(**fake)
    print("kernel output", out.shape, out.dtype)
